# revision 12
# baseline (speedup 1.0000x reference)
"""GQA attention (B=2, S=2048, D=2048, H=32, G=8, hd=64) on 8 TRN2 cores.

Sharding: 2 batch groups x 4 TP ranks. Core c: batch b=c//4, rank r=c%4.
Each rank owns 2 KV groups (8 Q heads). All compute in bf16 (f32 PSUM accum).

Layout strategy (all transposes done on host):
  - x^T resident in SBUF; projections produce Q^T/K^T [feat, tok] and
    V [tok, feat] directly, so scores S^T [k, q] come out transpose-free and
    P^T blocks feed the PV matmul as the stationary operand with no on-chip
    transposes at all.
  - softmax denominator: V is augmented with a ones column, so the PV matmul
    accumulates sum_k(exp) in PSUM row 64 for free. 1/d via exp(-ln(d)).
  - per-rank o^T [512, 2048] + recip rows are AllGathered over the TP group;
    each rank then output-projects its own 512-token slice (division folded
    into the lhsT of the output projection).
"""

import sys

sys.path.insert(0, "/opt/trn_rl_repo")

import numpy as np
import ml_dtypes

import concourse.bass as bass
import concourse.tile as tile
from concourse import bacc, mybir
from concourse.bass_utils import run_bass_kernel_spmd

BF16 = ml_dtypes.bfloat16
B, S, D = 2, 2048, 2048
H, G, HD = 32, 8, 64
REP = H // G
N_CORES = 8
TP = 4
QF = 512   # q features per rank
KF = 128   # k/v features per rank
DC = D // 128  # 16 dim chunks
TOK = S // TP  # 512 output tokens per rank
RG = [[0, 1, 2, 3], [4, 5, 6, 7]]

_CACHE = {}


def _build():
    f32 = mybir.dt.float32
    bf16 = mybir.dt.bfloat16
    nc = bacc.Bacc("TRN2", target_bir_lowering=False, debug=False, num_devices=N_CORES)

    xt = nc.dram_tensor("xt", [128, DC, S], bf16, kind="ExternalInput").ap()
    wqt = nc.dram_tensor("wqt", [128, DC, QF], bf16, kind="ExternalInput").ap()
    wkt = nc.dram_tensor("wkt", [128, DC, KF], bf16, kind="ExternalInput").ap()
    wvt = nc.dram_tensor("wvt", [128, DC, 130], bf16, kind="ExternalInput").ap()
    cosr = nc.dram_tensor("cosr", [128, S], bf16, kind="ExternalInput").ap()
    sinr = nc.dram_tensor("sinr", [128, S], bf16, kind="ExternalInput").ap()
    wot = nc.dram_tensor("wot", [128, DC, 512], bf16, kind="ExternalInput").ap()
    out = nc.dram_tensor("out", [S, 512], f32, kind="ExternalOutput").ap()

    Exp = mybir.ActivationFunctionType.Exp
    Ln = mybir.ActivationFunctionType.Ln
    swap_mask = [i ^ 1 for i in range(32)]

    from contextlib import ExitStack
    with tile.TileContext(nc) as tc, ExitStack() as ctx:
        consts = ctx.enter_context(tc.tile_pool(name="consts", bufs=1))
        qk = ctx.enter_context(tc.tile_pool(name="qk", bufs=1))
        io = ctx.enter_context(tc.tile_pool(name="io", bufs=2))
        work = ctx.enter_context(tc.tile_pool(name="work", bufs=3))
        psum = ctx.enter_context(tc.tile_pool(name="psum", bufs=4, space="PSUM"))
        opsum = ctx.enter_context(tc.tile_pool(name="opsum", bufs=4, space="PSUM"))
        dram = ctx.enter_context(tc.tile_pool(name="dram", bufs=1, space="DRAM"))

        # ---- load inputs
        xt_sb = qk.tile([128, DC, S], bf16, tag="xt")
        nc.sync.dma_start(out=xt_sb[:], in_=xt[:])
        wqt_sb = consts.tile([128, DC, QF], bf16, tag="wqt")
        nc.sync.dma_start(out=wqt_sb[:], in_=wqt[:])
        wkt_sb = consts.tile([128, DC, KF], bf16, tag="wkt")
        nc.sync.dma_start(out=wkt_sb[:], in_=wkt[:])
        wvt_sb = consts.tile([128, DC, 130], bf16, tag="wvt")
        nc.sync.dma_start(out=wvt_sb[:], in_=wvt[:])
        cos_sb = consts.tile([128, S], bf16, tag="cos")
        nc.sync.dma_start(out=cos_sb[:], in_=cosr[:])
        sin_sb = consts.tile([128, S], bf16, tag="sin")
        nc.sync.dma_start(out=sin_sb[:], in_=sinr[:])

        # ---- projections: Q^T (4 j-tiles), K^T, V(+ones)
        qt_sb = qk.tile([128, 4, S], bf16, tag="qt")
        kt_sb = qk.tile([128, S], bf16, tag="kt")
        vaug_sb = qk.tile([128, DC, 130], bf16, tag="vaug")

        for j in range(4):
            for nt in range(4):
                ps = psum.tile([128, 512], mybir.dt.float32, tag="ps")
                for c in range(DC):
                    nc.tensor.matmul(
                        ps,
                        lhsT=wqt_sb[:, c, j * 128:(j + 1) * 128],
                        rhs=xt_sb[:, c, nt * 512:(nt + 1) * 512],
                        start=(c == 0),
                        stop=(c == DC - 1),
                    )
                nc.vector.tensor_copy(qt_sb[:, j, nt * 512:(nt + 1) * 512], ps)
        for nt in range(4):
            ps = psum.tile([128, 512], mybir.dt.float32, tag="ps")
            for c in range(DC):
                nc.tensor.matmul(
                    ps,
                    lhsT=wkt_sb[:, c, :],
                    rhs=xt_sb[:, c, nt * 512:(nt + 1) * 512],
                    start=(c == 0),
                    stop=(c == DC - 1),
                )
            nc.vector.tensor_copy(kt_sb[:, nt * 512:(nt + 1) * 512], ps)
        for tb in range(DC):
            ps = psum.tile([128, 512], mybir.dt.float32, tag="ps")
            for c in range(DC):
                nc.tensor.matmul(
                    ps[:, 0:130],
                    lhsT=xt_sb[:, c, tb * 128:(tb + 1) * 128],
                    rhs=wvt_sb[:, c, :],
                    start=(c == 0),
                    stop=(c == DC - 1),
                )
            nc.vector.tensor_copy(vaug_sb[:, tb, :], ps[:, 0:130])
        nc.vector.memset(vaug_sb[:, :, 64:65], 1.0)
        nc.vector.memset(vaug_sb[:, :, 129:130], 1.0)

        # ---- RoPE on Q^T and K^T (pair-swap shuffle + signed sin table)
        rope_tiles = [qt_sb[:, j, :] for j in range(4)] + [kt_sb[:, :]]
        for t in rope_tiles:
            sw = work.tile([128, S], bf16, tag="rsw")
            nc.vector.stream_shuffle(sw, t, swap_mask)
            nc.vector.tensor_mul(sw, sw, sin_sb[:])
            tmp = work.tile([128, S], bf16, tag="rtmp")
            nc.vector.tensor_mul(tmp, t, cos_sb[:])
            nc.vector.tensor_add(t, sw, tmp)

        # ---- attention
        # denom slots: head lh -> partition 32*(lh%4), free block lh//4
        denom_sb = consts.tile([97, 2 * S], mybir.dt.float32, tag="denom")
        nc.vector.memset(denom_sb[:], 1.0)
        ag1_in = dram.tile([512, S], bf16)
        scale = float(1.0 / np.sqrt(HD))

        for j in range(4):
            for qb in range(4):
                qsl = slice(qb * 512, (qb + 1) * 512)
                oA = opsum.tile([65, 512], mybir.dt.float32, tag="ops")
                oB = opsum.tile([65, 512], mybir.dt.float32, tag="ops")
                for kb in range(DC):
                    ksl = slice(kb * 128, (kb + 1) * 128)
                    sA = psum.tile([128, 512], mybir.dt.float32, tag="ps")
                    sB = psum.tile([128, 512], mybir.dt.float32, tag="ps")
                    nc.tensor.matmul(
                        sA, lhsT=kt_sb[0:64, ksl], rhs=qt_sb[0:64, j, qsl],
                        start=True, stop=True, tile_position=(0, 0),
                    )
                    nc.tensor.matmul(
                        sB, lhsT=kt_sb[64:128, ksl], rhs=qt_sb[64:128, j, qsl],
                        start=True, stop=True, tile_position=(64, 0),
                    )
                    pA = work.tile([128, 512], bf16, tag="pa")
                    pB = work.tile([128, 512], bf16, tag="pb")
                    nc.scalar.activation(pA, sA, Exp, scale=scale)
                    nc.scalar.activation(pB, sB, Exp, scale=scale)
                    nc.tensor.matmul(
                        oA, lhsT=vaug_sb[:, kb, 0:65], rhs=pA,
                        start=(kb == 0), stop=(kb == DC - 1),
                    )
                    nc.tensor.matmul(
                        oB, lhsT=vaug_sb[:, kb, 65:130], rhs=pB,
                        start=(kb == 0), stop=(kb == DC - 1),
                    )
                otA = work.tile([64, 512], bf16, tag="ot")
                nc.vector.tensor_copy(otA, oA[0:64, :])
                nc.sync.dma_start(out=ag1_in[j * 64:(j + 1) * 64, qsl], in_=otA)
                otB = work.tile([64, 512], bf16, tag="ot")
                nc.vector.tensor_copy(otB, oB[0:64, :])
                nc.sync.dma_start(out=ag1_in[(4 + j) * 64:(5 + j) * 64, qsl], in_=otB)
                nc.vector.tensor_copy(
                    denom_sb[32 * j:32 * j + 1, qsl], oA[64:65, :])
                nc.vector.tensor_copy(
                    denom_sb[32 * j:32 * j + 1, S + qb * 512:S + (qb + 1) * 512],
                    oB[64:65, :])

        # reciprocal of denominators: 1/d = exp(-ln(d))
        nc.scalar.activation(denom_sb[:], denom_sb[:], Ln)
        nc.scalar.activation(denom_sb[:], denom_sb[:], Exp, scale=-1.0)

        # ---- AllGather o^T and recips across the TP group
        # ag2_in rows = local heads 0..7; slot s holds [head s | head s+4]
        ag2_in = dram.tile([8, S], mybir.dt.float32)
        ag2_r = ag2_in.rearrange("(a b) t -> b a t", a=2)
        for s in range(4):
            nc.sync.dma_start(
                out=ag2_r[s], in_=denom_sb[32 * s:32 * s + 1, :])
        ag1_out = dram.tile([2048, S], bf16)
        ag2_out = dram.tile([32, S], mybir.dt.float32)
        nc.gpsimd.collective_compute(
            "AllGather", mybir.AluOpType.bypass, replica_groups=RG,
            ins=[ag1_in.opt()], outs=[ag1_out.opt()],
        )
        nc.gpsimd.collective_compute(
            "AllGather", mybir.AluOpType.bypass, replica_groups=RG,
            ins=[ag2_in.opt()], outs=[ag2_out.opt()],
        )

        # ---- output projection, sharded by OUTPUT FEATURES (rank-dependence
        # lives in the per-core wot input shard: wo.T[:, r*512:(r+1)*512]).
        # Every core projects ALL 2048 tokens onto its 512 output features.
        wot_sb = consts.tile([128, DC, 512], bf16, tag="wot")
        nc.sync.dma_start(out=wot_sb[:], in_=wot[:])
        # gathered o^T, chunk layout [p, c, t]; reuses the xt slot (xt is dead)
        ot_sb = qk.tile([128, DC, S], bf16, tag="xt")
        nc.sync.dma_start(
            out=ot_sb[:], in_=ag1_out.rearrange("(c p) t -> p c t", p=128)
        )
        for ic in range(DC):
            r2 = io.tile([128, S], bf16, tag="r2")
            nc.gpsimd.dma_start(
                out=r2[0:64, :],
                in_=ag2_out[2 * ic:2 * ic + 1, :].partition_broadcast(64),
            )
            nc.gpsimd.dma_start(
                out=r2[64:128, :],
                in_=ag2_out[2 * ic + 1:2 * ic + 2, :].partition_broadcast(64),
            )
            nc.vector.tensor_mul(ot_sb[:, ic, :], ot_sb[:, ic, :], r2)
        for tb in range(DC):
            ps = psum.tile([128, 512], mybir.dt.float32, tag="ps")
            for ic in range(DC):
                nc.tensor.matmul(
                    ps,
                    lhsT=ot_sb[:, ic, tb * 128:(tb + 1) * 128],
                    rhs=wot_sb[:, ic, :],
                    start=(ic == 0),
                    stop=(ic == DC - 1),
                )
            osb = work.tile([128, 512], mybir.dt.float32, tag="osb")
            nc.vector.tensor_copy(osb, ps)
            nc.sync.dma_start(out=out[tb * 128:(tb + 1) * 128, :], in_=osb)

    nc.compile()
    return nc


def _prep_inputs(x, freqs_cos, freqs_sin, wqkv, wo):
    """Build per-core input maps (host-side shard + transpose + bf16 cast)."""
    ins = []
    wo_t = np.ascontiguousarray(wo.T)  # [i, j]
    cos_h = np.empty((128, S), np.float32)
    sin_h = np.empty((128, S), np.float32)
    cs = freqs_cos[:, 0, :]  # [S, 64]
    sn = freqs_sin[:, 0, :]
    for p in range(128):
        cos_h[p] = cs[:, p % 64]
        sin_h[p] = sn[:, p % 64] * (-1.0 if p % 2 == 0 else 1.0)
    cos_h = cos_h.astype(BF16)
    sin_h = sin_h.astype(BF16)

    for c in range(N_CORES):
        b, r = divmod(c, TP)
        xt_h = np.ascontiguousarray(
            x[b].T.reshape(DC, 128, S).transpose(1, 0, 2)).astype(BF16)
        # Q rows, permuted: j-tile j = [head 8r+j | head 8r+4+j]
        rows = []
        for j in range(4):
            for h in (8 * r + j, 8 * r + 4 + j):
                rows.extend(range(h * HD, (h + 1) * HD))
        wq_sel = wqkv[rows, :]  # [512, D]
        wqt_h = np.ascontiguousarray(
            wq_sel.T.reshape(DC, 128, QF).transpose(1, 0, 2)).astype(BF16)
        krows = []
        for g in (2 * r, 2 * r + 1):
            krows.extend(range(H * HD + g * HD, H * HD + (g + 1) * HD))
        wk_sel = wqkv[krows, :]
        wkt_h = np.ascontiguousarray(
            wk_sel.T.reshape(DC, 128, KF).transpose(1, 0, 2)).astype(BF16)
        vrows = []
        for g in (2 * r, 2 * r + 1):
            vrows.extend(range((H + G) * HD + g * HD, (H + G) * HD + (g + 1) * HD))
        wv_sel = wqkv[vrows, :]  # [128, D]
        wvt_aug = np.zeros((D, 130), np.float32)
        wvt_aug[:, 0:64] = wv_sel[0:64].T
        wvt_aug[:, 65:129] = wv_sel[64:128].T
        wvt_h = np.ascontiguousarray(
            wvt_aug.reshape(DC, 128, 130).transpose(1, 0, 2)).astype(BF16)
        wot_h = np.ascontiguousarray(
            wo_t[:, r * 512:(r + 1) * 512]
            .reshape(DC, 128, 512).transpose(1, 0, 2)).astype(BF16)
        ins.append({
            "xt": xt_h, "wqt": wqt_h, "wkt": wkt_h, "wvt": wvt_h,
            "cosr": cos_h, "sinr": sin_h, "wot": wot_h,
        })
    return ins


TRACE = False


def kernel(x, freqs_cos, freqs_sin, wqkv, wo):
    if "nc" not in _CACHE:
        _CACHE["nc"] = _build()
    nc = _CACHE["nc"]
    ins = _prep_inputs(x, freqs_cos, freqs_sin, wqkv, wo)
    res = run_bass_kernel_spmd(nc, ins, list(range(N_CORES)), trace=TRACE)
    _CACHE["res"] = res
    out = np.empty((B, S, D), np.float32)
    for c in range(N_CORES):
        b, r = divmod(c, TP)
        out[b, :, r * 512:(r + 1) * 512] = res.results[c]["out"]
    return out


if __name__ == "__main__":
    rng = np.random.default_rng(0)
    x = rng.normal(size=(B, S, D)).astype(np.float32)
    fc = rng.random(size=(S, 1, HD)).astype(np.float32)
    fs = rng.random(size=(S, 1, HD)).astype(np.float32)
    wq = rng.normal(size=(3072, D)).astype(np.float32) * 0.02
    wo = rng.normal(size=(D, D)).astype(np.float32) * 0.02
    o = kernel(x, fc, fs, wq, wo)
    print(o.shape, o.dtype)
